# revision 22
# baseline (speedup 1.0000x reference)
"""Multi-head attention (B=2, N=2048, d_model=1024, 16 heads x 64) on 8
Trainium2 NeuronCores.

Sharding: batch x head-group. Core c handles batch b = c//4 and heads
4*(c%4) .. 4*(c%4)+3. Projection weights are column-sliced (rows for Wo) so
each core computes q/k/v projections only for its 4 heads, full attention
for those heads, and a partial output projection. The host sums the four
partial outputs per batch (tensor-parallel reduce on to_out) and adds bo.

Device kernel (per core), matmuls in fp32r (rne-11 mantissa):
  qT/kT : projections producing [head-dim, seq] (lhsT = W chunk)
  v     : natural [seq, head-dim] with a ones column folded in (M=65)
  ST    : k^T q per head -> scores^T [keys, queries]; K=64 row-tile PAIRS
          (two heads concurrently on PE tiles T0/T8)
  E     : exp(ST * scale) via ScalarE eviction PSUM->SBUF (the wall:
          ~1 elem/lane/cycle regardless of dtype)
  AV    : [v|ones]^T @ E -> [65, 512]: rows 0-63 = O^T, row 64 = denom
  norm  : one batched reciprocal per query block, denom broadcast via a
          K=4 pattern matmul, DVE multiply into O^T
  out   : O^T-as-lhsT @ Wo slice -> partial [2048, 1024]
"""

import numpy as np

import concourse.mybir as mybir
import concourse.tile as tile
from concourse import bacc
from concourse import bass_utils
from concourse.tile_rust import add_dep_helper

F32 = mybir.dt.float32
F32R = mybir.dt.float32r
EXP = mybir.ActivationFunctionType.Exp

B = 2
N = 2048
D_MODEL = 1024
NHEAD = 16
DIM_HEAD = 64
SCALE = DIM_HEAD ** (-0.5)
N_CORES = 8
HEADS_PER_CORE = 4          # 2 pairs
INNER = HEADS_PER_CORE * DIM_HEAD  # 256

QB = 512                    # query block
N_QB = N // QB              # 4
N_KC = N // 128             # 16 key chunks


def _rne11(x: np.ndarray) -> np.ndarray:
    """Round fp32 to fp32r (round-to-nearest-even, 11 mantissa bits)."""
    b = np.ascontiguousarray(x, dtype=np.float32).view(np.uint32)
    lsb = (b >> np.uint32(12)) & np.uint32(1)
    r = (b + np.uint32(0x7FF) + lsb) & np.uint32(0xFFFFF000)
    return r.view(np.float32)


def build_nc():
    nc = bacc.Bacc("TRN2", target_bir_lowering=False, debug=False,
                   num_devices=N_CORES)
    xqt = nc.dram_tensor("xqt", [D_MODEL, N], F32R, kind="ExternalInput").ap()
    xkt = nc.dram_tensor("xkt", [D_MODEL, N], F32R, kind="ExternalInput").ap()
    xvt = nc.dram_tensor("xvt", [D_MODEL, N], F32R, kind="ExternalInput").ap()
    wq = nc.dram_tensor("wq", [D_MODEL, INNER], F32R, kind="ExternalInput").ap()
    wk = nc.dram_tensor("wk", [D_MODEL, INNER], F32R, kind="ExternalInput").ap()
    wv = nc.dram_tensor("wv", [D_MODEL, INNER], F32R, kind="ExternalInput").ap()
    wo = nc.dram_tensor("wo", [INNER, D_MODEL], F32R, kind="ExternalInput").ap()
    vones = nc.dram_tensor("vones", [128, N_KC, HEADS_PER_CORE, 1], F32R,
                           kind="ExternalInput").ap()
    # bc pattern: pat4[k, p, m] = 1 where head k owns output rows m in pair p
    pat4 = nc.dram_tensor("pat4", [128, 2, 128], F32R, kind="ExternalInput").ap()
    out = nc.dram_tensor("out", [N, D_MODEL], F32, kind="ExternalOutput").ap()

    with tile.TileContext(nc) as tc:
        with (
            tc.tile_pool(name="wpool", bufs=1) as wpool,
            tc.tile_pool(name="persist", bufs=1) as persist,
            tc.tile_pool(name="xin", bufs=3) as xin,
            tc.tile_pool(name="ering", bufs=9) as ering,
            tc.tile_pool(name="stage", bufs=3) as stage,
            tc.tile_pool(name="ps_st", bufs=2, space="PSUM") as ps_st,
            tc.tile_pool(name="ps_av", bufs=1, space="PSUM") as ps_av,
            tc.tile_pool(name="ps_misc", bufs=2, space="PSUM") as ps_misc,
        ):
            # ---- weights on sync queue, ordered by first use ----
            wk_sb = wpool.tile([128, 8, INNER], F32R)
            nc.sync.dma_start(wk_sb[:], wk.rearrange("(c p) m -> p c m", p=128))
            wq_sb = wpool.tile([128, 8, INNER], F32R)
            nc.sync.dma_start(wq_sb[:], wq.rearrange("(c p) m -> p c m", p=128))

            qt_sb = persist.tile([128, 2, N], F32R)
            kt_sb = persist.tile([128, 2, N], F32R)
            v_sb = persist.tile([128, N_KC, HEADS_PER_CORE, DIM_HEAD + 1], F32R)
            ot_sb = persist.tile([128, 2, N], F32R)

            xqt_r = xqt.rearrange("(c p) n -> p c n", p=128)
            xkt_r = xkt.rearrange("(c p) n -> p c n", p=128)
            xvt_r = xvt.rearrange("(c p) n -> p c n", p=128)

            def emit_kt(n):
                ns = slice(n * QB, (n + 1) * QB)
                xk_t = xin.tile([128, 8, QB], F32R, tag="xin", name=f"xk_{n}")
                nc.scalar.dma_start(xk_t[:], xkt_r[:, :, ns])
                for m in range(2):
                    pk = ps_misc.tile([128, QB], F32, tag="mp", name=f"pk{n}{m}")
                    for c in range(8):
                        nc.tensor.matmul(
                            pk[:], wk_sb[:, c, m * 128:(m + 1) * 128],
                            xk_t[:, c, :], start=(c == 0), stop=(c == 7))
                    nc.vector.tensor_copy(kt_sb[:, m, ns], pk[:])

            def emit_qt(n):
                ns = slice(n * QB, (n + 1) * QB)
                xq_t = xin.tile([128, 8, QB], F32R, tag="xin", name=f"xq_{n}")
                nc.sync.dma_start(xq_t[:], xqt_r[:, :, ns])
                for m in range(2):
                    pq = ps_misc.tile([128, QB], F32, tag="mp", name=f"pq{n}{m}")
                    for c in range(8):
                        nc.tensor.matmul(
                            pq[:], wq_sb[:, c, m * 128:(m + 1) * 128],
                            xq_t[:, c, :], start=(c == 0), stop=(c == 7))
                    nc.vector.tensor_copy(qt_sb[:, m, ns], pq[:])

            def emit_vblock(n):
                ns = slice(n * QB, (n + 1) * QB)
                xv_t = xin.tile([128, 8, QB], F32R, tag="xin", name=f"xv_{n}")
                nc.scalar.dma_start(xv_t[:], xvt_r[:, :, ns])
                for kci in range(4):
                    kc = n * 4 + kci
                    kcs = slice(kci * 128, (kci + 1) * 128)
                    pva = ps_misc.tile([128, INNER], F32, tag="mp",
                                       padded_shape=[128, 512], name=f"pva{kc}")
                    pvb = ps_misc.tile([128, INNER], F32, tag="mp",
                                       padded_shape=[128, 512], name=f"pvb{kc}")
                    for c in range(8):
                        nc.tensor.matmul(
                            pva[:], xv_t[0:64, c, kcs],
                            wv_sb[0:64, c, :], start=(c == 0), stop=(c == 7))
                        nc.tensor.matmul(
                            pvb[:], xv_t[64:128, c, kcs],
                            wv_sb[64:128, c, :], start=(c == 0), stop=(c == 7))
                    va_sb = stage.tile([128, INNER], F32, tag="vasb",
                                       name=f"va{kc}", bufs=2)
                    nc.vector.tensor_copy(va_sb[:], pva[:])
                    nc.vector.scalar_tensor_tensor(
                        v_sb[:, kc, :, 0:DIM_HEAD],
                        pvb[:].rearrange("p (h d) -> p h d", h=HEADS_PER_CORE),
                        1.0,
                        va_sb[:].rearrange("p (h d) -> p h d", h=HEADS_PER_CORE),
                        mybir.AluOpType.mult, mybir.AluOpType.add)

            def emit_outproj_chunk(qb, idx):
                qc = qb * 4 + idx // 2
                dc = idx % 2
                cs = slice(qc * 128, (qc + 1) * 128)
                op = ps_misc.tile([128, 512], F32, tag="mp", name=f"op{qc}{dc}")
                for ic in range(2):
                    nc.tensor.matmul(
                        op[:], ot_sb[:, ic, cs],
                        wo_sb[:, ic, dc * 512:(dc + 1) * 512],
                        start=(ic == 0), stop=(ic == 1))
                o_stage = stage.tile([128, 512], F32, tag="ostage",
                                     name=f"ost{qc}{dc}", bufs=2)
                nc.vector.tensor_copy(o_stage[:], op[:])
                nc.sync.dma_start(out[cs, dc * 512:(dc + 1) * 512], o_stage[:])

            qb_state = {}

            def emit_st(qb, p, kc):
                qs = slice(qb * QB, (qb + 1) * QB)
                ks = slice(kc * 128, (kc + 1) * 128)
                st = ps_st.tile([128, 1024], F32, tag="st", name=f"st{qb}{p}{kc}")
                mm0 = nc.tensor.matmul(st[:, 0:512], kt_sb[0:64, p, ks],
                                       qt_sb[0:64, p, qs], start=True, stop=True)
                nc.tensor.matmul(st[:, 512:1024], kt_sb[64:128, p, ks],
                                 qt_sb[64:128, p, qs], start=True, stop=True)
                e_t = ering.tile([128, 2, 512], F32R, tag="e",
                                 name=f"e{qb}{p}{kc}")
                nc.scalar.activation(
                    e_t[:], st[:].rearrange("p (h n) -> p h n", h=2),
                    EXP, scale=float(SCALE))
                return e_t, mm0

            def emit_av(qb, p, kc, avs, e_t):
                for hh in range(2):
                    nc.tensor.matmul(
                        avs[hh][0:DIM_HEAD + 1, :],
                        v_sb[:, kc, 2 * p + hh, :], e_t[:, hh, :],
                        start=(kc == 0), stop=(kc == N_KC - 1))

            def end_pair(qb, p, avs, order_after=None):
                den4 = qb_state[qb]["den4"]
                qs = slice(qb * QB, (qb + 1) * QB)
                avsb = []
                for hh in range(2):
                    a_sb = stage.tile([DIM_HEAD + 1, 512], F32, tag="avsb",
                                      name=f"avsb{qb}_{p}_{hh}", bufs=4)
                    nc.vector.tensor_copy(a_sb[:], avs[hh][0:DIM_HEAD + 1, :])
                    k32 = 32 * (2 * p + hh)
                    nc.vector.tensor_copy(den4[k32:k32 + 1, :],
                                          a_sb[DIM_HEAD:DIM_HEAD + 1, :])
                    avsb.append(a_sb)
                rec = stage.tile([128, 512], F32, tag="rec",
                                 name=f"rec{qb}{p}", bufs=2)
                with nc.allow_low_precision(reason="softmax denom recip"):
                    nc.vector.reciprocal(rec[:], den4[:])
                recr = stage.tile([128, 512], F32R, tag="recr",
                                  name=f"recr{qb}{p}", bufs=2)
                nc.vector.tensor_copy(recr[:], rec[:])
                bc = ps_misc.tile([128, 512], F32, tag="mp", name=f"bc{qb}{p}")
                bcmm = nc.tensor.matmul(bc[:], pat_sb[:, p, :], recr[:],
                                        start=True, stop=True)
                if order_after is not None:
                    add_dep_helper(order_after.ins, bcmm.ins, sync=False,
                                   reason="hold bc behind ST stream")
                for hh in range(2):
                    nc.vector.tensor_mul(
                        ot_sb[hh * 64:(hh + 1) * 64, p, qs],
                        avsb[hh][0:DIM_HEAD, :],
                        bc[hh * 64:(hh + 1) * 64, :])


            def begin_qb(qb):
                den4 = stage.tile([128, 512], F32, tag="den4", name=f"den{qb}",
                                  bufs=1)
                nc.vector.memset(den4[:], 1.0)
                qb_state[qb] = dict(den4=den4)

            def new_avs(qb, p):
                return [ps_av.tile([128, 512], F32, tag=f"av{hh}",
                                   name=f"av{hh}_{qb}_{p}")
                        for hh in range(2)]

            def phase_fillers(qb, p):
                f = []
                if qb == 0 and p == 0:
                    for n in range(1, N_QB):
                        f.append((4 * n - 1, lambda n=n: (emit_kt(n),
                                                          emit_vblock(n))))
                elif qb == 0 and p == 1:
                    f.append((7, lambda: emit_qt(1)))
                else:
                    prev = qb - 1
                    if p == 0:
                        for g in range(4):
                            f.append(((7, 9, 11, 13)[g],
                                      lambda g=g: emit_outproj_chunk(prev, g)))
                    else:
                        if qb < N_QB - 1:
                            f.append((5, lambda: emit_qt(qb + 1)))
                        for g in range(4):
                            f.append(((3, 7, 10, 13)[g],
                                      lambda g=g: emit_outproj_chunk(prev, 4 + g)))
                return dict(f)

            emit_kt(0)
            emit_qt(0)

            wv_sb = wpool.tile([128, 8, INNER], F32R)
            nc.sync.dma_start(wv_sb[:], wv.rearrange("(c p) m -> p c m", p=128))
            nc.sync.dma_start(v_sb[:, :, :, DIM_HEAD:DIM_HEAD + 1], vones[:])
            wo_sb = wpool.tile([128, 2, D_MODEL], F32R)
            nc.sync.dma_start(wo_sb[:], wo.rearrange("(c p) d -> p c d", p=128))
            pat_sb = wpool.tile([128, 2, 128], F32R)
            nc.sync.dma_start(pat_sb[:], pat4[:])

            emit_vblock(0)

            AV_LAG = 4
            phases = [(qb, p) for qb in range(N_QB) for p in range(2)]
            pending = None   # (qb, p, avs, [(kc, e_t)...])

            for qb, p in phases:
                if p == 0:
                    begin_qb(qb)
                avs = new_avs(qb, p)
                fillers = phase_fillers(qb, p)
                eq = []
                for kc in range(N_KC):
                    e_t, stmm = emit_st(qb, p, kc)
                    eq.append((kc, e_t))
                    if kc == AV_LAG - 1 and pending is not None:
                        pq, pp, pavs, peq = pending
                        for pkc, pe_t in peq:
                            emit_av(pq, pp, pkc, pavs, pe_t)
                        end_pair(pq, pp, pavs, order_after=stmm)
                        pending = None
                    if kc >= AV_LAG:
                        pkc, pe_t = eq[kc - AV_LAG]
                        emit_av(qb, p, pkc, avs, pe_t)
                    if kc in fillers:
                        fillers[kc]()
                pending = (qb, p, avs, eq[N_KC - AV_LAG:])

            pq, pp, pavs, peq = pending
            for pkc, pe_t in peq:
                emit_av(pq, pp, pkc, pavs, pe_t)
            end_pair(pq, pp, pavs)
            for idx in range(8):
                emit_outproj_chunk(N_QB - 1, idx)
    nc.compile()
    return nc


_NC_CACHE = None


def _get_nc():
    global _NC_CACHE
    if _NC_CACHE is None:
        _NC_CACHE = build_nc()
    return _NC_CACHE


def _make_pat4():
    pat = np.zeros((128, 2, 128), np.float32)
    for p in range(2):
        for hh in range(2):
            pat[32 * (2 * p + hh), p, hh * 64:(hh + 1) * 64] = 1.0
    return pat


def make_in_maps(query, key, value, Wq, Wk, Wv, Wo):
    query = np.asarray(query, np.float32)
    key = np.asarray(key, np.float32)
    value = np.asarray(value, np.float32)
    vones = np.ones((128, N_KC, HEADS_PER_CORE, 1), np.float32)
    pat4 = _make_pat4()
    in_maps = []
    for c in range(N_CORES):
        b = c // 4
        hg = c % 4
        cols = slice(hg * INNER, (hg + 1) * INNER)
        in_maps.append({
            "xqt": _rne11(np.asarray(query[b]).T),
            "xkt": _rne11(np.asarray(key[b]).T),
            "xvt": _rne11(np.asarray(value[b]).T),
            "wq": _rne11(np.asarray(Wq[:, cols])),
            "wk": _rne11(np.asarray(Wk[:, cols])),
            "wv": _rne11(np.asarray(Wv[:, cols])),
            "wo": _rne11(np.asarray(Wo[cols, :])),
            "vones": vones,
            "pat4": pat4,
        })
    return in_maps


def kernel(query, key, value, Wq, Wk, Wv, Wo, bo, _trace=False, _trace_cores=None):
    nc = _get_nc()
    in_maps = make_in_maps(query, key, value, Wq, Wk, Wv, Wo)
    res = bass_utils.run_bass_kernel_spmd(
        nc, in_maps, core_ids=list(range(N_CORES)), trace=_trace,
        trace_cores=_trace_cores)
    out = np.zeros((B, N, D_MODEL), np.float32)
    for c in range(N_CORES):
        out[c // 4] += res.results[c]["out"]
    out += np.asarray(bo, np.float32)[None, None, :]
    if _trace:
        return out, res
    return out
